# revision 43
# baseline (speedup 1.0000x reference)
"""Trainium2 Bass kernel for gnn_message_passing patch extraction.

Reference computation (see problem):
    P = (L - 128)//64 + 1 = 7811 patches over signals I, Q of length L=500000
    patches[p, t, c] = (I, Q)[c][p*64 + t]                       [P, 128, 2]
    adj = sigmoid(edge_weights) * band_mask(|i-j| in [1, 16])    [128, 128]
    adjs[p] = adj  (broadcast)                                   [P, 128, 128]

This is purely HBM-write-bandwidth bound: adjs is ~512 MB of a single
64 KB tile repeated P times.  Strategy (8 cores, data-parallel over
patches, 977 patches/core):

  - band mask folded into the input on host (ewb = ew - 1e4 off-band),
    so the device adj compute is a single ACT sigmoid
  - flatten adj to one partition, then broadcast it to a [128, 16384]
    SBUF tile via 32 K=1 PE matmuls against a ones vector (each
    partition then holds the full flattened adj matrix); tiny warmup
    matmuls ramp the PE while the adj chain latency plays out
  - stream the tile out in column-grouped 1 MB DMAs (8 groups x 8 row
    blocks, 8 KB contiguous per partition), group-major so streaming
    starts after the first 4 matmuls; the two HWDGE rings (SP/ACT)
    each own one contiguous 32 MB half of the output (measured best:
    ~350-400 GB/s sustained vs ~313 GB/s for monolithic 8 MB stores)
  - patches: strided window loads of I/Q (one DMA each), DVE
    interleave into [p, t, c] layout, one 1 MB contiguous store
"""

import sys

for _p in ("/opt/trn_rl_repo",):
    if _p not in sys.path:
        sys.path.insert(0, _p)

import numpy as np

import concourse.bass as bass
import concourse.bacc as bacc
import concourse.mybir as mybir
from concourse import tile
from concourse.bass_utils import run_bass_kernel_spmd

PL = 128        # patch length
STRIDE = 64
WS = 16         # band window
L = 500000
P_TOTAL = (L - PL) // STRIDE + 1   # 7811
N_CORES = 8
PPC = 977       # patches per core (8*977 = 7816 >= 7811)
PBLK = 8        # patch-path blocks of 128 (8*128 = 1024 rows, trimmed to 977 on host)
# I/Q elements each core needs: patch 1023's window ends at 1023*64+128 = 65600
ILEN = 1023 * STRIDE + PL          # 65600 floats per core (padded slice of I/Q)
ADJ_BLOCKS = [128] * 7 + [81]      # 977 adjs rows per core

_CACHE: dict = {}


def _build_program(store_reps: int = 1, col_groups: int = 8, warmups: int = 24,
                   rings: int = 2, ring_assign: str = "bhalf",
                   store_order: str = "g", bench_internal: bool = False):
    f32 = mybir.dt.float32
    nc = bacc.Bacc("TRN2", target_bir_lowering=False, debug=False)

    i_in = nc.dram_tensor("i_in", [ILEN], f32, kind="ExternalInput").ap()
    q_in = nc.dram_tensor("q_in", [ILEN], f32, kind="ExternalInput").ap()
    # ewb = edge_weights with -1e4 added outside the band (host-side const
    # fold); sigmoid(ewb) is then exactly the banded adjacency.
    ewb = nc.dram_tensor("ewb", [PL, PL], f32, kind="ExternalInput").ap()
    # sel[k, k0*128+p] = (k == k0): one-hot selectors for the broadcast
    # matmuls (constant, host-built)
    sel = nc.dram_tensor("sel", [16, 16 * PL], f32, kind="ExternalInput").ap()
    # bench_internal: identical device work, but big outputs stay in device
    # DRAM (timing runs only; avoids shipping 0.5 GB per call)
    out_kind = "Internal" if bench_internal else "ExternalOutput"
    patches_out = nc.dram_tensor(
        "patches_out", [PBLK * 128, PL * 2], f32, kind=out_kind
    ).ap()
    adjs_out = nc.dram_tensor(
        "adjs_out", [PPC, PL * PL], f32, kind=out_kind
    ).ap()
    if bench_internal:
        token_out = nc.dram_tensor(
            "token_out", [1, 16], f32, kind="ExternalOutput"
        ).ap()

    with tile.TileContext(nc) as tc:
        with (
            tc.tile_pool(name="const", bufs=1) as cpool,
            tc.tile_pool(name="psum", bufs=7, space="PSUM") as ppool,
        ):
            # ---------------- adj chain (critical path; emit first) ----------
            # load ewb in a [16, 1024] row-major-flat layout: partition k
            # holds adj rows 8k..8k+7 flattened, so the broadcast matmuls can
            # read their rhs slices directly — no flatten DMA on the path
            ew_t = cpool.tile([16, 8 * PL], f32)
            nc.sync.dma_start(
                out=ew_t[:, :], in_=ewb.rearrange("(k r) b -> k (r b)", k=16)
            )
            sel_t = cpool.tile([16, 16 * PL], f32)
            nc.scalar.dma_start(out=sel_t[:, :], in_=sel[:, :])

            ones_t = cpool.tile([1, PL], f32)
            nc.vector.memset(ones_t[:], 1.0)
            dummy_t = cpool.tile([1, 64], f32)
            nc.gpsimd.memset(dummy_t[:], 0.0)

            # PE HAM warmup: tiny matmuls into a scratch bank while the adj
            # chain latency plays out, so the real broadcast runs at speed.
            warm_ps = ppool.tile([128, 64], f32, tag="warm_ps", bufs=1)
            for _ in range(warmups):
                nc.tensor.matmul(
                    out=warm_ps[:], lhsT=ones_t[0:1, :], rhs=dummy_t[0:1, :],
                    start=True, stop=True,
                )

            sig_t = cpool.tile([16, 8 * PL], f32)
            nc.scalar.activation(
                sig_t[:], ew_t[:], mybir.ActivationFunctionType.Sigmoid
            )

            # broadcast flat adj to all 128 partitions via K=16 selector
            # matmuls: out[p, n] = sum_k sel[k, k0*128+p] * sig[k, n]
            #                    = sig[k0, n]   (exact: 0*x terms vanish)
            big = cpool.tile([128, PL * PL], f32)   # 64 KB per partition
            for c in range(PL * PL // 512):
                k0 = c // 2
                mm_ps = ppool.tile([128, 512], f32, tag="mm_ps")
                nc.tensor.matmul(
                    out=mm_ps[:],
                    lhsT=sel_t[:, k0 * PL:(k0 + 1) * PL],
                    rhs=sig_t[0:16, (c % 2) * 512:(c % 2) * 512 + 512],
                    start=True,
                    stop=True,
                )
                nc.vector.tensor_copy(
                    out=big[:, c * 512:(c + 1) * 512], in_=mm_ps[:]
                )

            # ---------------- adjs stores: the 64 MB stream ------------------
            # column-grouped: group g covers adj rows [g*gr, (g+1)*gr) and can
            # start as soon as its slice of `big` is built.
            hwdge = [nc.sync, nc.scalar, nc.gpsimd][:rings]
            if col_groups == 0:              # "ramp": tiny first groups so
                gsizes = [512, 512, 1024] + [2048] * 7   # streaming starts
            else:                            # after ONE matmul; 2048-col
                gsizes = [PL * PL // col_groups] * col_groups  # steady state
            goff = np.cumsum([0] + gsizes).tolist()
            assert goff[-1] == PL * PL
            row_of = np.cumsum([0] + ADJ_BLOCKS).tolist()
            ngroups = len(gsizes)
            # partial 81-row block first within each group: the kernel's
            # last DMAs are then full-width (all 16 SDMA engine ports busy)
            border = [len(ADJ_BLOCKS) - 1] + list(range(len(ADJ_BLOCKS) - 1))
            if store_order == "b":           # ascending address within ring
                schedule = [(g, bi) for bi in border for g in range(ngroups)]
            else:                            # group-major: earliest start
                schedule = [(g, bi) for g in range(ngroups) for bi in border]
            q = 0
            for _rep in range(store_reps):   # >1 only for benchmarking
                for g, bi in schedule:
                    n = ADJ_BLOCKS[bi]
                    row = row_of[bi]
                    dst = adjs_out[row:row + n, goff[g]:goff[g + 1]]
                    if ring_assign == "g":
                        ring = g
                    elif ring_assign == "bhalf":
                        ring = 0 if bi < len(ADJ_BLOCKS) // 2 else 1
                    else:
                        ring = q
                    hwdge[ring % len(hwdge)].dma_start(
                        out=dst, in_=big[0:n, goff[g]:goff[g + 1]]
                    )
                    q += 1

            # ---------------- patches path (fills engine gaps) ---------------
            i_tile = cpool.tile([128, PBLK * PL], f32)
            q_tile = cpool.tile([128, PBLK * PL], f32)
            # overlapping windows: src[p, b, t] = in[(b*128 + p)*64 + t]
            # on gpsimd (SWDGE): these APs need ~1k descriptors of 512 B, and
            # the Pool engine is idle here while the HWDGE rings (SP/ACT)
            # carry the flatten + the 64 MB store stream
            win_ap = [[STRIDE, 128], [128 * STRIDE, PBLK], [1, PL]]
            iload = nc.gpsimd.dma_start(
                out=i_tile[:].rearrange("p (b t) -> p b t", b=PBLK),
                in_=bass.AP(i_in.tensor, 0, win_ap),
            )
            qload = nc.gpsimd.dma_start(
                out=q_tile[:].rearrange("p (b t) -> p b t", b=PBLK),
                in_=bass.AP(q_in.tensor, 0, win_ap),
            )
            del iload, qload

            pt = cpool.tile([128, PBLK * PL * 2], f32)
            ptv = pt[:].rearrange("p (b t c) -> p b t c", b=PBLK, c=2)
            itv = i_tile[:].rearrange("p (b t) -> p b t", b=PBLK)
            qtv = q_tile[:].rearrange("p (b t) -> p b t", b=PBLK)
            for b in range(PBLK):
                nc.vector.tensor_copy(out=ptv[:, b, :, 0], in_=itv[:, b, :])
                nc.vector.tensor_copy(out=ptv[:, b, :, 1], in_=qtv[:, b, :])
            nc.gpsimd.dma_start(
                out=patches_out.rearrange("(b p) f -> p b f", b=PBLK),
                in_=pt[:].rearrange("p (b f) -> p b f", b=PBLK),
            )

            if bench_internal:
                tok_t = cpool.tile([1, 16], f32)
                nc.vector.memset(tok_t[:], 1.0)
                nc.sync.dma_start(out=token_out[:, :], in_=tok_t[:])

    nc.compile()
    return nc


def _get_program():
    if "nc" not in _CACHE:
        _CACHE["nc"] = _build_program()
    return _CACHE["nc"]


def _band_bias() -> np.ndarray:
    """0 inside the band (0 < |i-j| <= WS), -1e4 outside: sigmoid(ew + bias)
    saturates to exactly 0 off-band."""
    r = np.arange(PL)
    d = np.abs(r[:, None] - r[None, :])
    band = (d <= WS) & (d > 0)
    return np.where(band, np.float32(0), np.float32(-1e4))


def _selectors() -> np.ndarray:
    """sel[k, k0*128+p] = 1.0 iff k == k0 (one-hot rows for the broadcast)."""
    sel = np.zeros((16, 16 * PL), np.float32)
    for k0 in range(16):
        sel[k0, k0 * PL:(k0 + 1) * PL] = 1.0
    return sel


def kernel(I, Q, edge_weights):
    I = np.ascontiguousarray(np.asarray(I, dtype=np.float32))
    Q = np.ascontiguousarray(np.asarray(Q, dtype=np.float32))
    ew = np.asarray(edge_weights, dtype=np.float32)
    ewb = np.ascontiguousarray(ew + _band_bias()).astype(np.float32)
    sel = _selectors()

    in_maps = []
    for m in range(N_CORES):
        off = m * PPC * STRIDE
        i_sl = np.zeros(ILEN, np.float32)
        q_sl = np.zeros(ILEN, np.float32)
        seg = I[off:off + ILEN]
        i_sl[:seg.shape[0]] = seg
        seg = Q[off:off + ILEN]
        q_sl[:seg.shape[0]] = seg
        in_maps.append({"i_in": i_sl, "q_in": q_sl, "ewb": ewb, "sel": sel})

    nc = _get_program()
    res = run_bass_kernel_spmd(nc, in_maps, core_ids=list(range(N_CORES)))

    patches = np.concatenate(
        [r["patches_out"][:PPC] for r in res.results], axis=0
    )[:P_TOTAL].reshape(P_TOTAL, PL, 2)
    adjs = np.concatenate(
        [r["adjs_out"] for r in res.results], axis=0
    )[:P_TOTAL].reshape(P_TOTAL, PL, PL)
    return patches, adjs


# revision 51
# speedup vs baseline: 1.1440x; 1.1440x over previous
"""Trainium2 Bass kernel for gnn_message_passing patch extraction.

Reference computation (see problem):
    P = (L - 128)//64 + 1 = 7811 patches over signals I, Q of length L=500000
    patches[p, t, c] = (I, Q)[c][p*64 + t]                       [P, 128, 2]
    adj = sigmoid(edge_weights) * band_mask(|i-j| in [1, 16])    [128, 128]
    adjs[p] = adj  (broadcast)                                   [P, 128, 128]

This is purely HBM-write-bandwidth bound: adjs is ~512 MB of a single
64 KB tile repeated P times.  Strategy (8 cores, data-parallel over
patches, 977 patches/core):

  - band mask folded into the input on host (ewb = ew - 1e4 off-band),
    so the device adj compute is a single ACT sigmoid
  - flatten adj to one partition, then broadcast it to a [128, 16384]
    SBUF tile via 32 K=1 PE matmuls against a ones vector (each
    partition then holds the full flattened adj matrix); tiny warmup
    matmuls ramp the PE while the adj chain latency plays out
  - stream the tile out in column-grouped 1 MB DMAs (8 groups x 8 row
    blocks, 8 KB contiguous per partition), group-major so streaming
    starts after the first 4 matmuls; the two HWDGE rings (SP/ACT)
    each own one contiguous 32 MB half of the output (measured best:
    ~350-400 GB/s sustained vs ~313 GB/s for monolithic 8 MB stores)
  - patches: strided window loads of I/Q (one DMA each), DVE
    interleave into [p, t, c] layout, one 1 MB contiguous store
"""

import sys

for _p in ("/opt/trn_rl_repo",):
    if _p not in sys.path:
        sys.path.insert(0, _p)

import numpy as np

import concourse.bass as bass
import concourse.bacc as bacc
import concourse.mybir as mybir
from concourse import tile
from concourse.bass_utils import run_bass_kernel_spmd

PL = 128        # patch length
STRIDE = 64
WS = 16         # band window
L = 500000
P_TOTAL = (L - PL) // STRIDE + 1   # 7811
N_CORES = 8
PPC = 977       # patches per core (8*977 = 7816 >= 7811)
PBLK = 8        # patch-path blocks of 128 (8*128 = 1024 rows, trimmed to 977 on host)
# I/Q elements each core needs: patch 1023's window ends at 1023*64+128 = 65600
ILEN = 1023 * STRIDE + PL          # 65600 floats per core (padded slice of I/Q)
ADJ_BLOCKS = [128] * 7 + [81]      # 977 adjs rows per core

_CACHE: dict = {}


def _build_program(store_reps: int = 1, col_groups: int = 8, warmups: int = 8,
                   rings: int = 2, ring_assign: str = "bhalf",
                   store_order: str = "g", bench_internal: bool = False):
    f32 = mybir.dt.float32
    nc = bacc.Bacc("TRN2", target_bir_lowering=False, debug=False)

    i_in = nc.dram_tensor("i_in", [ILEN], f32, kind="ExternalInput").ap()
    q_in = nc.dram_tensor("q_in", [ILEN], f32, kind="ExternalInput").ap()
    # ewb = edge_weights with -1e4 added outside the band (host-side const
    # fold); sigmoid(ewb) is then exactly the banded adjacency.
    ewb = nc.dram_tensor("ewb", [PL, PL], f32, kind="ExternalInput").ap()
    # sel[k, c*128+p] = (k == c): one-hot selectors for the broadcast
    # matmuls (constant, host-built)
    sel = nc.dram_tensor("sel", [32, 32 * PL], f32, kind="ExternalInput").ap()
    # bench_internal: identical device work, but big outputs stay in device
    # DRAM (timing runs only; avoids shipping 0.5 GB per call)
    out_kind = "Internal" if bench_internal else "ExternalOutput"
    patches_out = nc.dram_tensor(
        "patches_out", [PBLK * 128, PL * 2], f32, kind=out_kind
    ).ap()
    adjs_out = nc.dram_tensor(
        "adjs_out", [PPC, PL * PL], f32, kind=out_kind
    ).ap()
    if bench_internal:
        token_out = nc.dram_tensor(
            "token_out", [1, 16], f32, kind="ExternalOutput"
        ).ap()

    with tile.TileContext(nc) as tc:
        with (
            tc.tile_pool(name="const", bufs=1) as cpool,
            tc.tile_pool(name="psum", bufs=7, space="PSUM") as ppool,
        ):
            # ---------------- adj chain (critical path; emit first) ----------
            # load ewb in a [32, 512] row-major-flat layout: partition c holds
            # flat adj cols c*512..(c+1)*512 — exactly matmul c's output
            # window, so no flatten DMA is needed anywhere
            ew_t = cpool.tile([32, 4 * PL], f32)
            nc.sync.dma_start(
                out=ew_t[:, :], in_=ewb.rearrange("(k r) b -> k (r b)", k=32)
            )
            # sel on the SWDGE ring: both HWDGE rings stay clear for stores
            sel_t = cpool.tile([32, 32 * PL], f32)
            nc.gpsimd.dma_start(out=sel_t[:, :], in_=sel[:, :])

            ones_t = cpool.tile([1, PL], f32)
            nc.vector.memset(ones_t[:], 1.0)
            dummy_t = cpool.tile([1, 64], f32)
            nc.gpsimd.memset(dummy_t[:], 0.0)

            # PE HAM warmup: tiny matmuls into a scratch bank while the adj
            # chain latency plays out, so the real broadcast runs at speed.
            warm_ps = ppool.tile([128, 64], f32, tag="warm_ps", bufs=1)
            for _ in range(warmups):
                nc.tensor.matmul(
                    out=warm_ps[:], lhsT=ones_t[0:1, :], rhs=dummy_t[0:1, :],
                    start=True, stop=True,
                )

            sig_t = cpool.tile([32, 4 * PL], f32)
            nc.scalar.activation(
                sig_t[:], ew_t[:], mybir.ActivationFunctionType.Sigmoid
            )

            # broadcast flat adj to all 128 partitions via K=16 selector
            # matmuls: out[p, n] = sum_k sel[k, k0*128+p] * sig[k, n]
            #                    = sig[k0, n]   (exact: 0*x terms vanish)
            big = cpool.tile([128, PL * PL], f32)   # 64 KB per partition
            big_copies = []
            for c in range(PL * PL // 512):
                mm_ps = ppool.tile([128, 512], f32, tag="mm_ps")
                nc.tensor.matmul(
                    out=mm_ps[:],
                    lhsT=sel_t[:, c * PL:(c + 1) * PL],
                    rhs=sig_t[0:32, :],
                    start=True,
                    stop=True,
                )
                big_copies.append(nc.vector.tensor_copy(
                    out=big[:, c * 512:(c + 1) * 512], in_=mm_ps[:]
                ))

            # ---------------- adjs stores: the 64 MB stream ------------------
            # column-grouped: group g covers adj rows [g*gr, (g+1)*gr) and can
            # start as soon as its slice of `big` is built.
            hwdge = [nc.sync, nc.scalar, nc.gpsimd][:rings]
            if col_groups == 0:              # "ramp": tiny first groups so
                gsizes = [512, 512, 1024] + [2048] * 7   # streaming starts
            else:                            # after ONE matmul; 2048-col
                gsizes = [PL * PL // col_groups] * col_groups  # steady state
            goff = np.cumsum([0] + gsizes).tolist()
            assert goff[-1] == PL * PL
            row_of = np.cumsum([0] + ADJ_BLOCKS).tolist()
            ngroups = len(gsizes)
            # partial 81-row block first within each group: the kernel's
            # last DMAs are then full-width (all 16 SDMA engine ports busy)
            border = [len(ADJ_BLOCKS) - 1] + list(range(len(ADJ_BLOCKS) - 1))
            if store_order == "b":           # ascending address within ring
                schedule = [(g, bi) for bi in border for g in range(ngroups)]
            else:                            # group-major: earliest start
                schedule = [(g, bi) for g in range(ngroups) for bi in border]
            q = 0
            for _rep in range(store_reps):   # >1 only for benchmarking
                for g, bi in schedule:
                    n = ADJ_BLOCKS[bi]
                    row = row_of[bi]
                    dst = adjs_out[row:row + n, goff[g]:goff[g + 1]]
                    if ring_assign == "g":
                        ring = g
                    elif ring_assign == "bhalf":
                        ring = 0 if bi < len(ADJ_BLOCKS) // 2 else 1
                    else:
                        ring = q
                    hwdge[ring % len(hwdge)].dma_start(
                        out=dst, in_=big[0:n, goff[g]:goff[g + 1]]
                    )
                    q += 1

            # ---------------- patches path (fills engine gaps) ---------------
            i_tile = cpool.tile([128, PBLK * PL], f32)
            q_tile = cpool.tile([128, PBLK * PL], f32)
            # overlapping windows: src[p, b, t] = in[(b*128 + p)*64 + t]
            # on gpsimd (SWDGE): these APs need ~1k descriptors of 512 B, and
            # the Pool engine is idle here while the HWDGE rings (SP/ACT)
            # carry the flatten + the 64 MB store stream
            win_ap = [[STRIDE, 128], [128 * STRIDE, PBLK], [1, PL]]
            iload = nc.gpsimd.dma_start(
                out=i_tile[:].rearrange("p (b t) -> p b t", b=PBLK),
                in_=bass.AP(i_in.tensor, 0, win_ap),
            )
            qload = nc.gpsimd.dma_start(
                out=q_tile[:].rearrange("p (b t) -> p b t", b=PBLK),
                in_=bass.AP(q_in.tensor, 0, win_ap),
            )
            del iload, qload

            pt = cpool.tile([128, PBLK * PL * 2], f32)
            ptv = pt[:].rearrange("p (b t c) -> p b t c", b=PBLK, c=2)
            itv = i_tile[:].rearrange("p (b t) -> p b t", b=PBLK)
            qtv = q_tile[:].rearrange("p (b t) -> p b t", b=PBLK)
            for b in range(PBLK):
                ic = nc.vector.tensor_copy(out=ptv[:, b, :, 0],
                                           in_=itv[:, b, :])
                qc = nc.vector.tensor_copy(out=ptv[:, b, :, 1],
                                           in_=qtv[:, b, :])
                # keep DVE free for the group-0 PSUM->SBUF copies that gate
                # the first store; the interleaves have ~180 us of slack
                for cp in (ic, qc):
                    tile.add_dep_helper(
                        cp.ins, big_copies[3].ins, sync=False,
                        reason="defer patch interleave behind group-0 copies",
                    )
            nc.gpsimd.dma_start(
                out=patches_out.rearrange("(b p) f -> p b f", b=PBLK),
                in_=pt[:].rearrange("p (b f) -> p b f", b=PBLK),
            )

            if bench_internal:
                tok_t = cpool.tile([1, 16], f32)
                nc.vector.memset(tok_t[:], 1.0)
                nc.sync.dma_start(out=token_out[:, :], in_=tok_t[:])

    nc.compile()
    return nc


def _get_program():
    if "nc" not in _CACHE:
        _CACHE["nc"] = _build_program()
    return _CACHE["nc"]


def _band_bias() -> np.ndarray:
    """0 inside the band (0 < |i-j| <= WS), -1e4 outside: sigmoid(ew + bias)
    saturates to exactly 0 off-band."""
    r = np.arange(PL)
    d = np.abs(r[:, None] - r[None, :])
    band = (d <= WS) & (d > 0)
    return np.where(band, np.float32(0), np.float32(-1e4))


def _selectors() -> np.ndarray:
    """sel[k, c*128+p] = 1.0 iff k == c (one-hot rows for the broadcast)."""
    sel = np.zeros((32, 32 * PL), np.float32)
    for c in range(32):
        sel[c, c * PL:(c + 1) * PL] = 1.0
    return sel


def kernel(I, Q, edge_weights):
    I = np.ascontiguousarray(np.asarray(I, dtype=np.float32))
    Q = np.ascontiguousarray(np.asarray(Q, dtype=np.float32))
    ew = np.asarray(edge_weights, dtype=np.float32)
    ewb = np.ascontiguousarray(ew + _band_bias()).astype(np.float32)
    sel = _selectors()

    in_maps = []
    for m in range(N_CORES):
        off = m * PPC * STRIDE
        i_sl = np.zeros(ILEN, np.float32)
        q_sl = np.zeros(ILEN, np.float32)
        seg = I[off:off + ILEN]
        i_sl[:seg.shape[0]] = seg
        seg = Q[off:off + ILEN]
        q_sl[:seg.shape[0]] = seg
        in_maps.append({"i_in": i_sl, "q_in": q_sl, "ewb": ewb, "sel": sel})

    nc = _get_program()
    res = run_bass_kernel_spmd(nc, in_maps, core_ids=list(range(N_CORES)))

    patches = np.concatenate(
        [r["patches_out"][:PPC] for r in res.results], axis=0
    )[:P_TOTAL].reshape(P_TOTAL, PL, 2)
    adjs = np.concatenate(
        [r["adjs_out"] for r in res.results], axis=0
    )[:P_TOTAL].reshape(P_TOTAL, PL, PL)
    return patches, adjs


# revision 55
# speedup vs baseline: 1.1543x; 1.0090x over previous
"""Trainium2 Bass kernel for gnn_message_passing patch extraction.

Reference computation (see problem):
    P = (L - 128)//64 + 1 = 7811 patches over signals I, Q of length L=500000
    patches[p, t, c] = (I, Q)[c][p*64 + t]                       [P, 128, 2]
    adj = sigmoid(edge_weights) * band_mask(|i-j| in [1, 16])    [128, 128]
    adjs[p] = adj  (broadcast)                                   [P, 128, 128]

This is purely HBM-write-bandwidth bound: adjs is ~512 MB of a single
64 KB tile repeated P times.  Strategy (8 cores, data-parallel over
patches, 977 patches/core):

  - band mask folded into the input on host (ewb = ew - 1e4 off-band),
    so the device adj compute is a single ACT sigmoid
  - ewb loaded in a [32, 512] row-major-flat layout (partition c = flat
    adj cols c*512..): sigmoid runs there in 0.4 us and 32 K=32
    selector matmuls (one-hot host constant picks the row — exact in
    fp) broadcast it to a [128, 16384] SBUF tile, no flatten DMA
    anywhere; tiny warmup matmuls ramp the PE during the load latency
  - stream the tile out in column-grouped 1 MB DMAs (8 groups x 8 row
    blocks, 8 KB contiguous per partition), group-major so streaming
    starts after the first 4 matmuls; the two HWDGE rings (SP/ACT)
    each own one contiguous 32 MB half of the output (measured best:
    ~400-417 GB/s sustained vs ~313 GB/s for monolithic 8 MB stores)
  - patches: strided window loads of I/Q (one DMA each), DVE
    interleave into [p, t, c] layout, one 1 MB contiguous store
"""

import sys

for _p in ("/opt/trn_rl_repo",):
    if _p not in sys.path:
        sys.path.insert(0, _p)

import numpy as np

import concourse.bass as bass
import concourse.bacc as bacc
import concourse.mybir as mybir
from concourse import tile
from concourse.bass_utils import run_bass_kernel_spmd

PL = 128        # patch length
STRIDE = 64
WS = 16         # band window
L = 500000
P_TOTAL = (L - PL) // STRIDE + 1   # 7811
N_CORES = 8
PPC = 977       # patches per core (8*977 = 7816 >= 7811)
PBLK = 8        # patch-path blocks of 128 (8*128 = 1024 rows, trimmed to 977 on host)
# I/Q elements each core needs: patch 1023's window ends at 1023*64+128 = 65600
ILEN = 1023 * STRIDE + PL          # 65600 floats per core (padded slice of I/Q)
ADJ_BLOCKS = [128] * 7 + [81]      # 977 adjs rows per core

_CACHE: dict = {}


def _build_program(store_reps: int = 1, col_groups: int = 8, warmups: int = 8,
                   rings: int = 2, ring_assign: str = "bhalf",
                   store_order: str = "g", bench_internal: bool = False):
    f32 = mybir.dt.float32
    nc = bacc.Bacc("TRN2", target_bir_lowering=False, debug=False)

    i_in = nc.dram_tensor("i_in", [ILEN], f32, kind="ExternalInput").ap()
    q_in = nc.dram_tensor("q_in", [ILEN], f32, kind="ExternalInput").ap()
    # ewb = edge_weights with -1e4 added outside the band (host-side const
    # fold); sigmoid(ewb) is then exactly the banded adjacency.
    ewb = nc.dram_tensor("ewb", [PL, PL], f32, kind="ExternalInput").ap()
    # sel[k, c*128+p] = (k == c): one-hot selectors for the broadcast
    # matmuls (constant, host-built)
    sel = nc.dram_tensor("sel", [32, 32 * PL], f32, kind="ExternalInput").ap()
    # bench_internal: identical device work, but big outputs stay in device
    # DRAM (timing runs only; avoids shipping 0.5 GB per call)
    out_kind = "Internal" if bench_internal else "ExternalOutput"
    patches_out = nc.dram_tensor(
        "patches_out", [PBLK * 128, PL * 2], f32, kind=out_kind
    ).ap()
    adjs_out = nc.dram_tensor(
        "adjs_out", [PPC, PL * PL], f32, kind=out_kind
    ).ap()
    if bench_internal:
        token_out = nc.dram_tensor(
            "token_out", [1, 16], f32, kind="ExternalOutput"
        ).ap()

    with tile.TileContext(nc) as tc:
        with (
            tc.tile_pool(name="const", bufs=1) as cpool,
            tc.tile_pool(name="psum", bufs=7, space="PSUM") as ppool,
        ):
            # ---------------- adj chain (critical path; emit first) ----------
            # load ewb in a [32, 512] row-major-flat layout: partition c holds
            # flat adj cols c*512..(c+1)*512 — exactly matmul c's output
            # window, so no flatten DMA is needed anywhere
            ew_t = cpool.tile([32, 4 * PL], f32)
            nc.sync.dma_start(
                out=ew_t[:, :], in_=ewb.rearrange("(k r) b -> k (r b)", k=32)
            )
            # sel on the SWDGE ring: both HWDGE rings stay clear for stores
            sel_t = cpool.tile([32, 32 * PL], f32)
            nc.gpsimd.dma_start(out=sel_t[:, :], in_=sel[:, :])

            ones_t = cpool.tile([1, PL], f32)
            nc.vector.memset(ones_t[:], 1.0)
            dummy_t = cpool.tile([1, 64], f32)
            nc.gpsimd.memset(dummy_t[:], 0.0)

            # PE HAM warmup: tiny matmuls into a scratch bank while the adj
            # chain latency plays out, so the real broadcast runs at speed.
            warm_ps = ppool.tile([128, 64], f32, tag="warm_ps", bufs=1)
            for _ in range(warmups):
                nc.tensor.matmul(
                    out=warm_ps[:], lhsT=ones_t[0:1, :], rhs=dummy_t[0:1, :],
                    start=True, stop=True,
                )

            sig_t = cpool.tile([32, 4 * PL], f32)
            nc.scalar.activation(
                sig_t[:], ew_t[:], mybir.ActivationFunctionType.Sigmoid
            )

            # broadcast flat adj to all 128 partitions via K=16 selector
            # matmuls: out[p, n] = sum_k sel[k, k0*128+p] * sig[k, n]
            #                    = sig[k0, n]   (exact: 0*x terms vanish)
            big = cpool.tile([128, PL * PL], f32)   # 64 KB per partition
            big_copies = []
            for c in range(PL * PL // 512):
                mm_ps = ppool.tile([128, 512], f32, tag="mm_ps")
                nc.tensor.matmul(
                    out=mm_ps[:],
                    lhsT=sel_t[:, c * PL:(c + 1) * PL],
                    rhs=sig_t[0:32, :],
                    start=True,
                    stop=True,
                )
                big_copies.append(nc.vector.tensor_copy(
                    out=big[:, c * 512:(c + 1) * 512], in_=mm_ps[:]
                ))

            # ---------------- adjs stores: the 64 MB stream ------------------
            # column-grouped: group g covers adj rows [g*gr, (g+1)*gr) and can
            # start as soon as its slice of `big` is built.
            hwdge = [nc.sync, nc.scalar, nc.gpsimd][:rings]
            if col_groups == 0:              # "ramp": halved first groups so
                gsizes = [1024, 1024] + [2048] * 7       # streaming starts
            else:                            # after TWO matmuls; 2048-col
                gsizes = [PL * PL // col_groups] * col_groups  # steady state
            goff = np.cumsum([0] + gsizes).tolist()
            assert goff[-1] == PL * PL
            row_of = np.cumsum([0] + ADJ_BLOCKS).tolist()
            ngroups = len(gsizes)
            # partial 81-row block first within each group: the kernel's
            # last DMAs are then full-width (all 16 SDMA engine ports busy)
            border = [len(ADJ_BLOCKS) - 1] + list(range(len(ADJ_BLOCKS) - 1))
            if store_order == "b":           # ascending address within ring
                schedule = [(g, bi) for bi in border for g in range(ngroups)]
            else:                            # group-major: earliest start
                schedule = [(g, bi) for g in range(ngroups) for bi in border]
            q = 0
            for _rep in range(store_reps):   # >1 only for benchmarking
                for g, bi in schedule:
                    n = ADJ_BLOCKS[bi]
                    row = row_of[bi]
                    dst = adjs_out[row:row + n, goff[g]:goff[g + 1]]
                    if ring_assign == "g":
                        ring = g
                    elif ring_assign == "bhalf":
                        ring = 0 if bi < len(ADJ_BLOCKS) // 2 else 1
                    else:
                        ring = q
                    hwdge[ring % len(hwdge)].dma_start(
                        out=dst, in_=big[0:n, goff[g]:goff[g + 1]]
                    )
                    q += 1

            # ---------------- patches path (fills engine gaps) ---------------
            i_tile = cpool.tile([128, PBLK * PL], f32)
            q_tile = cpool.tile([128, PBLK * PL], f32)
            # overlapping windows: src[p, b, t] = in[(b*128 + p)*64 + t]
            # on gpsimd (SWDGE): these APs need ~1k descriptors of 512 B, and
            # the Pool engine is idle here while the HWDGE rings (SP/ACT)
            # carry the flatten + the 64 MB store stream
            win_ap = [[STRIDE, 128], [128 * STRIDE, PBLK], [1, PL]]
            iload = nc.gpsimd.dma_start(
                out=i_tile[:].rearrange("p (b t) -> p b t", b=PBLK),
                in_=bass.AP(i_in.tensor, 0, win_ap),
            )
            qload = nc.gpsimd.dma_start(
                out=q_tile[:].rearrange("p (b t) -> p b t", b=PBLK),
                in_=bass.AP(q_in.tensor, 0, win_ap),
            )
            del iload, qload

            pt = cpool.tile([128, PBLK * PL * 2], f32)
            ptv = pt[:].rearrange("p (b t c) -> p b t c", b=PBLK, c=2)
            itv = i_tile[:].rearrange("p (b t) -> p b t", b=PBLK)
            qtv = q_tile[:].rearrange("p (b t) -> p b t", b=PBLK)
            for b in range(PBLK):
                ic = nc.vector.tensor_copy(out=ptv[:, b, :, 0],
                                           in_=itv[:, b, :])
                qc = nc.vector.tensor_copy(out=ptv[:, b, :, 1],
                                           in_=qtv[:, b, :])
                # keep DVE free for the group-0 PSUM->SBUF copies that gate
                # the first store; the interleaves have ~180 us of slack
                for cp in (ic, qc):
                    tile.add_dep_helper(
                        cp.ins, big_copies[3].ins, sync=False,
                        reason="defer patch interleave behind group-0 copies",
                    )
            nc.gpsimd.dma_start(
                out=patches_out.rearrange("(b p) f -> p b f", b=PBLK),
                in_=pt[:].rearrange("p (b f) -> p b f", b=PBLK),
            )

            if bench_internal:
                tok_t = cpool.tile([1, 16], f32)
                nc.vector.memset(tok_t[:], 1.0)
                nc.sync.dma_start(out=token_out[:, :], in_=tok_t[:])

    nc.compile()
    return nc


def _get_program():
    if "nc" not in _CACHE:
        _CACHE["nc"] = _build_program()
    return _CACHE["nc"]


def _band_bias() -> np.ndarray:
    """0 inside the band (0 < |i-j| <= WS), -1e4 outside: sigmoid(ew + bias)
    saturates to exactly 0 off-band."""
    r = np.arange(PL)
    d = np.abs(r[:, None] - r[None, :])
    band = (d <= WS) & (d > 0)
    return np.where(band, np.float32(0), np.float32(-1e4))


def _selectors() -> np.ndarray:
    """sel[k, c*128+p] = 1.0 iff k == c (one-hot rows for the broadcast)."""
    sel = np.zeros((32, 32 * PL), np.float32)
    for c in range(32):
        sel[c, c * PL:(c + 1) * PL] = 1.0
    return sel


def kernel(I, Q, edge_weights):
    I = np.ascontiguousarray(np.asarray(I, dtype=np.float32))
    Q = np.ascontiguousarray(np.asarray(Q, dtype=np.float32))
    ew = np.asarray(edge_weights, dtype=np.float32)
    ewb = np.ascontiguousarray(ew + _band_bias()).astype(np.float32)
    sel = _selectors()

    in_maps = []
    for m in range(N_CORES):
        off = m * PPC * STRIDE
        i_sl = np.zeros(ILEN, np.float32)
        q_sl = np.zeros(ILEN, np.float32)
        seg = I[off:off + ILEN]
        i_sl[:seg.shape[0]] = seg
        seg = Q[off:off + ILEN]
        q_sl[:seg.shape[0]] = seg
        in_maps.append({"i_in": i_sl, "q_in": q_sl, "ewb": ewb, "sel": sel})

    nc = _get_program()
    res = run_bass_kernel_spmd(nc, in_maps, core_ids=list(range(N_CORES)))

    patches = np.concatenate(
        [r["patches_out"][:PPC] for r in res.results], axis=0
    )[:P_TOTAL].reshape(P_TOTAL, PL, 2)
    adjs = np.concatenate(
        [r["adjs_out"] for r in res.results], axis=0
    )[:P_TOTAL].reshape(P_TOTAL, PL, PL)
    return patches, adjs
